# revision 41
# baseline (speedup 1.0000x reference)
"""Single-head attention (nn_MultiHeadAttention) Trainium2 Bass kernel.

Full inputs: x [4, 2048, 1024], Wq/Wk/Wv/Wo [1024, 1024], biases [1024].
reference:  q = x @ Wq.T + bq ; k,v likewise
            scores = (q @ k.T) / sqrt(1024) ; attn = softmax(scores, -1)
            out = (attn @ v) @ Wo.T + bo

Sharding: 8 cores = 4 batches x 2 query-halves; each core owns 1024
queries and all 2048 keys of its batch (global key order everywhere).

Algebraic fusions (host-side weight transforms):
  scores:  q k^T = x (Wq^T Wk) x^T + (bq Wk) x^T + per-query consts that
           cancel in softmax.  A = Wq^T Wk is precomputed on the host, so
           the K projection disappears; the per-key offset o_k = x_k.(bq Wk)
           rides in through the exp's per-partition bias.
  output:  (attn @ (x Wv^T + bv)) Wo^T + bo = attn @ (x (Wo Wv)^T + bc)
           with bc = Wo bv + bo, because the softmax rows sum to 1.  With
           Wvo = Wo Wv precomputed on the host, the ctx matmul yields the
           FINAL output directly — no separate out-projection phase.

V dedup: each core projects VO' = x (Wo Wv)^T + bc only for its OWN 1024
keys (which equal its own query rows, passed as the separate xq input so
the program stays SPMD-uniform), then the core pair exchanges halves via
a pairwise AllGather through a DRAM bounce, hidden behind QA + scores.

Per-core pipeline (all matmul operands bf16, fp32 PSUM accumulation):
  VO phase:  VO'[s,f]  = xq^T Wvo^T + bc     (own keys, d-outer, first)
             spill -> AllGather[pair] -> reload full VO'   (async)
  QA phase:  QAT[d',q] = A^T xq^T            (d-outer)
  scores:    u[k,q]    = exp((QAT^T x)^T * scale + o_k * scale)
             Z[q]      = sum_k u             (DVE accumulation + gpsimd
                                              cross-partition all-reduce)
  out:       out[f,q]  = (VO'^T u) * (1/Z)   (stored f-major, host untiles)
"""

import numpy as np
from contextlib import ExitStack

import ml_dtypes

import concourse.bass as bass
import concourse.bacc as bacc
import concourse.bass_isa as bass_isa
import concourse.mybir as mybir
import concourse.tile as tile
from concourse import bass_utils

F32 = mybir.dt.float32
F32R = mybir.dt.float32r
BF16 = mybir.dt.bfloat16
AF = mybir.ActivationFunctionType
ALU = mybir.AluOpType

B, S, D = 4, 2048, 1024
SQ = S // 2  # queries per core
N_CORES = 8
SCALE = 1.0 / float(np.sqrt(D))

# matmul operand dtypes (PSUM accumulation is always fp32)
G1DT = BF16   # x, A, qa, wvo  (QA / scores / VO matmuls)
G2DT = BF16   # vo, u          (ctx matmuls)


def build_nc():
    P = 128
    DT = D // P          # contraction tiles (8)
    ET = D // P          # output-dim tiles (8)
    SQW = 512            # query free-dim block
    SQB = SQ // SQW      # (2)
    SKT = S // P         # key tiles (16)
    SOT = SQ // P        # own-key tiles (8)
    NBW = 512            # free-dim block over D for the VO phase
    NB = D // NBW        # (2)

    nc = bacc.Bacc("TRN2", target_bir_lowering=False, debug=False,
                   num_devices=N_CORES)

    # all inputs pre-tiled on the host so every DMA chunk is one contiguous
    # DRAM run (strided row-chunks cap DMA throughput on descriptor overhead)
    xTt = nc.dram_tensor("xTt", [DT, P, S], G1DT, kind="ExternalInput")
    xqt = nc.dram_tensor("xqt", [DT, P, SQ], G1DT, kind="ExternalInput")
    aMt = nc.dram_tensor("aMt", [DT, P, D], G1DT, kind="ExternalInput")
    wvoTt = nc.dram_tensor("wvoTt", [DT, P, D], G1DT, kind="ExternalInput")
    bcd = nc.dram_tensor("bc", [D], F32, kind="ExternalInput")
    soffd = nc.dram_tensor("soff", [S], F32, kind="ExternalInput")
    outd = nc.dram_tensor("out", [ET, SQB, P, SQW], F32, kind="ExternalOutput")

    def bcast_ap(handle):
        a = handle[:]
        return bass.AP(tensor=a.tensor, offset=a.offset, ap=[[0, P]] + list(a.ap))

    with tile.TileContext(nc) as tc, ExitStack() as top:
        psum = top.enter_context(tc.tile_pool(name="psum", bufs=8, space="PSUM"))
        dram = top.enter_context(tc.tile_pool(name="dram", bufs=1, space="DRAM"))
        singles = top.enter_context(tc.tile_pool(name="singles", bufs=1))
        vb_in = dram.tile([SOT, P, D], G2DT, name="vb_in", tag="vb_in")
        vb_out = dram.tile([2, SOT, P, D], G2DT, name="vb_out", tag="vb_out")

        # ---- right-side pools, reserved in release order (LIFO top last)
        v_pool = tc.alloc_tile_pool(name="v", bufs=SKT, side="right")
        v_tiles = [v_pool.tile([P, D], G2DT, name=f"v{i}", tag="v")
                   for i in range(SKT)]
        u_pool = tc.alloc_tile_pool(name="u", bufs=SKT * SQB, side="right")
        u_tiles = [[None] * SKT for _ in range(SQB)]
        zacc_pool = tc.alloc_tile_pool(name="zacc", bufs=SQB, side="right")
        wv_pool = tc.alloc_tile_pool(name="wv", bufs=1, side="right")
        vown_pool = tc.alloc_tile_pool(name="vown", bufs=SOT, side="right")

        # ---- left-side: xt/xq under qa under a_row (released in reverse)
        xt_pool = tc.alloc_tile_pool(name="xt", bufs=DT)
        xq_pool = tc.alloc_tile_pool(name="xq", bufs=DT)
        qa_pool = tc.alloc_tile_pool(name="qa", bufs=ET)
        qa_tiles = [qa_pool.tile([P, SQ], G1DT, name=f"qa{i}", tag="qa")
                    for i in range(ET)]
        a_pool = tc.alloc_tile_pool(name="arow", bufs=DT)

        # DMA plan: scalar stays free for ACT work (PSUM evacuation); sync
        # carries xq -> xt -> VO spills; gpsimd carries wvo -> bc -> a ->
        # the AllGather -> VO reloads -> z round-trip.  Loads are emitted in
        # consumption order: the VO d-loop of sgroup 0 needs wvo[d] +
        # xq[d][:, 0:512] per d-step.
        wv_full = wv_pool.tile([P, DT, D], G1DT, name="wv", tag="wv")
        xq_tiles = []
        for d in range(DT):
            xq_t = xq_pool.tile([P, SQ], G1DT, name=f"xq{d}", tag="xq")
            nc.sync.dma_start(out=xq_t, in_=xqt[d])
            nc.gpsimd.dma_start(out=wv_full[:, d, :], in_=wvoTt[d])
            xq_tiles.append(xq_t)
        bc_bc = singles.tile([P, D], F32, name="bc_bc", tag="bc_bc")
        nc.gpsimd.dma_start(out=bc_bc, in_=bcast_ap(bcd))
        a_rows = []
        for d in range(DT):
            ar = a_pool.tile([P, D], G1DT, name=f"ar{d}", tag="ar")
            nc.gpsimd.dma_start(out=ar, in_=aMt[d])
            a_rows.append(ar)
        xt_tiles = []
        for t in range(DT):
            xt_t = xt_pool.tile([P, S], G1DT, name=f"xt{t}", tag="xt")
            nc.sync.dma_start(out=xt_t, in_=xTt[t])
            xt_tiles.append(xt_t)

        def xt_slice(d, lo, width):
            return xt_tiles[d][:, lo:lo + width]

        # constants (emitted after the start-critical loads)
        soff_pt = singles.tile([P, SKT], F32, name="soff_pt", tag="soff_pt")
        nc.gpsimd.dma_start(out=soff_pt, in_=soffd[:].rearrange("(t p) -> p t", p=P))
        rz_bc = singles.tile([P, SQ], F32, name="rz_bc", tag="rz_bc")

        # HAM pre-warm: the PE's clock gate sits at 4/8 until ~3.4us of
        # sustained matmul activity.  Dummy matmuls on zeroed scratch keep
        # the array busy through the initial DMA wait so the real stream
        # starts at full clock (first ~13 matmuls otherwise run at half
        # rate).  Sized to end just as the first input tiles land.
        warm_l = singles.tile([P, P], G1DT, name="warm_l", tag="warm_l")
        warm_r = singles.tile([P, SQW], G1DT, name="warm_r", tag="warm_r")
        nc.vector.memset(warm_l, 0.0)
        nc.vector.memset(warm_r, 0.0)
        wpsum = psum.tile([P, SQW], F32, name="mm", tag="mm")
        for _ in range(36):
            nc.tensor.matmul(wpsum, lhsT=warm_l, rhs=warm_r,
                             start=True, stop=True)

        # ---------------- VO phase first (own keys only, d-outer) ----------
        # so the pair exchange starts early and hides behind QA + scores
        vown_tiles = [vown_pool.tile([P, D], G2DT, name=f"vo{i}", tag="vo")
                      for i in range(SOT)]
        for sg in range(2):
            pv = [psum.tile([P, NBW], F32, name="mm", tag="mm") for _ in range(8)]
            for d in range(DT):
                for si in range(4):
                    for eb in range(NB):
                        nc.tensor.matmul(
                            pv[si * 2 + eb],
                            lhsT=xq_tiles[d][:, (sg * 4 + si) * P:(sg * 4 + si + 1) * P],
                            rhs=wv_full[:, d, eb * NBW:(eb + 1) * NBW],
                            start=(d == 0), stop=(d == DT - 1),
                        )
            for si in range(4):
                for eb in range(NB):
                    s = sg * 4 + si
                    nc.vector.scalar_tensor_tensor(
                        out=vown_tiles[s][:, eb * NBW:(eb + 1) * NBW],
                        in0=pv[si * 2 + eb], scalar=1.0,
                        in1=bc_bc[:, eb * NBW:(eb + 1) * NBW],
                        op0=ALU.mult, op1=ALU.add,
                    )
            for si in range(4):
                s = sg * 4 + si
                nc.sync.dma_start(out=vb_in[s], in_=vown_tiles[s])
        # pairwise exchange: AllGather the spilled halves, reload both.
        nc.gpsimd.collective_compute(
            "AllGather",
            ALU.bypass,
            replica_groups=[[0, 1], [2, 3], [4, 5], [6, 7]],
            ins=[vb_in[:]],
            outs=[vb_out[:]],
        )
        for sk in range(SKT):
            nc.gpsimd.dma_start(out=v_tiles[sk], in_=vb_out[sk // SOT, sk % SOT])

        # ---------------- QA phase (d-outer) ----------------
        for sb in range(SQB):
            pq = [psum.tile([P, SQW], F32, name="mm", tag="mm") for _ in range(ET)]
            for d in range(DT):
                for et in range(ET):
                    nc.tensor.matmul(
                        pq[et],
                        lhsT=a_rows[d][:, et * P:(et + 1) * P],
                        rhs=xq_tiles[d][:, sb * SQW:(sb + 1) * SQW],
                        start=(d == 0), stop=(d == DT - 1),
                    )
            for et in range(ET):
                nc.scalar.activation(
                    out=qa_tiles[et][:, sb * SQW:(sb + 1) * SQW],
                    in_=pq[et], func=AF.Copy,
                )
        a_pool.release()

        # ---------------- scores + Z ----------------
        for sk in range(SKT):
            for q in range(SQB):
                ps = psum.tile([P, SQW], F32, name="mm", tag="mm")
                for e in range(ET):
                    nc.tensor.matmul(
                        ps,
                        lhsT=xt_slice(e, sk * P, P),
                        rhs=qa_tiles[e][:, q * SQW:(q + 1) * SQW],
                        start=(e == 0), stop=(e == ET - 1),
                    )
                ut = u_pool.tile([P, SQW], G2DT, name=f"u{q}_{sk}", tag="u")
                nc.scalar.activation(
                    out=ut, in_=ps, func=AF.Exp,
                    bias=soff_pt[:, sk:sk + 1], scale=SCALE,
                )
                u_tiles[q][sk] = ut
                if sk == 0:
                    za = zacc_pool.tile([P, SQW], F32R, name=f"za{q}", tag="za")
                    nc.vector.tensor_copy(za, ut)
                    if q == 0:
                        zacc = [za]
                    else:
                        zacc.append(za)
                else:
                    nc.vector.tensor_tensor(
                        out=zacc[q], in0=zacc[q], in1=ut, op=ALU.add)

        # Z -> 1/Z replicated across partitions, entirely off the PE queue:
        # gpsimd cross-partition all-reduce, then a DVE reciprocal
        for zq in range(SQB):
            zsum = singles.tile([P, SQW], F32, name=f"zsum{zq}", tag=f"zsum{zq}")
            nc.gpsimd.partition_all_reduce(
                zsum[:], zacc[zq][:], P, bass_isa.ReduceOp.add)
            nc.vector.reciprocal(
                out=rz_bc[:, zq * SQW:(zq + 1) * SQW], in_=zsum)

        vown_pool.release()
        wv_pool.release()
        zacc_pool.release()
        qa_pool.release()
        xq_pool.release()
        xt_pool.release()

        # ---------------- fused ctx/out phase ----------------
        with tc.tile_pool(name="ofly", bufs=4) as o_pool:
            for q in range(SQB):
                for e in range(ET):
                    pc = psum.tile([P, SQW], F32, name="mm", tag="mm")
                    for sk in range(SKT):
                        nc.tensor.matmul(
                            pc,
                            lhsT=v_tiles[sk][:, e * P:(e + 1) * P],
                            rhs=u_tiles[q][sk],
                            start=(sk == 0), stop=(sk == SKT - 1),
                        )
                    osb = o_pool.tile([P, SQW], F32, name="osb", tag="ofly")
                    last = (q == SQB - 1 and e == ET - 1)
                    # split the final block across two store queues so the
                    # kernel's last store drains sooner
                    hs = [(0, SQW // 2, nc.scalar), (SQW // 2, SQW, nc.sync)] \
                        if last else [(0, SQW, nc.scalar)]
                    for lo, hi, eng in hs:
                        nc.vector.tensor_tensor(
                            out=osb[:, lo:hi], in0=pc[:, lo:hi],
                            in1=rz_bc[:, q * SQW + lo:q * SQW + hi],
                            op=ALU.mult)
                        eng.dma_start(out=outd[e, q, :, lo:hi],
                                      in_=osb[:, lo:hi])
        u_pool.release()
        v_pool.release()

    nc.compile()
    return nc


_NC_CACHE = {}


def _get_nc():
    if "nc" not in _NC_CACHE:
        _NC_CACHE["nc"] = build_nc()
    return _NC_CACHE["nc"]


def _round_f32r(a):
    """Round-to-nearest to fp32r precision (fp22 = s1e8m13)."""
    u = np.ascontiguousarray(a, np.float32).view(np.uint32)
    u = ((u.astype(np.uint64) + 0x200) & 0xFFFFFC00).astype(np.uint32)
    return u.view(np.float32)


def _cast(a, dt):
    a = np.ascontiguousarray(np.asarray(a, np.float32))
    if dt == BF16:
        return a.astype(ml_dtypes.bfloat16)
    if dt == F32R:
        return _round_f32r(a)
    return a


def _tile_rows(m, dt):
    """[D, N] -> contiguous [D//128, 128, N] row-tiles, cast to dt."""
    m = np.asarray(m, np.float32)
    return np.ascontiguousarray(_cast(m, dt).reshape(m.shape[0] // 128, 128, -1))


def make_in_maps(x, Wq, bq, Wk, bk, Wv, bv, Wo, bo):
    x = np.asarray(x, np.float32)
    Wq = np.asarray(Wq, np.float32)
    Wk = np.asarray(Wk, np.float32)
    Wv = np.asarray(Wv, np.float32)
    Wo = np.asarray(Wo, np.float32)
    # A = Wq^T Wk so scores = x A x^T (+ per-key offset from bq, see header)
    aMt = _tile_rows(Wq.T @ Wk, G1DT)
    # Wvo = Wo Wv folds the output projection into the value path; the
    # matching bias constant is bc = Wo bv + bo (softmax rows sum to 1)
    wvoTt = _tile_rows((Wo @ Wv).T, G1DT)
    bc = np.ascontiguousarray(Wo @ np.asarray(bv, np.float32)
                              + np.asarray(bo, np.float32))
    ck = np.asarray(bq, np.float32) @ Wk  # [d]

    in_maps = []
    for c in range(N_CORES):
        b, h = c // 2, c % 2
        xb = x[b]  # [S, D], global key order
        own = xb[h * SQ:(h + 1) * SQ]
        xTt_c = _tile_rows(xb.T, G1DT)
        xqt_c = _tile_rows(own.T, G1DT)
        soff = np.ascontiguousarray((xb @ ck) * np.float32(SCALE))
        in_maps.append({
            "xTt": xTt_c, "xqt": xqt_c, "aMt": aMt, "wvoTt": wvoTt,
            "bc": bc, "soff": soff,
        })
    return in_maps


def assemble(results):
    out = np.empty((B, S, D), np.float32)
    for c in range(N_CORES):
        b, h = c // 2, c % 2
        # [8(e), 2(qb), 128(f), 512(q)] tiled, f-major -> [1024 q, 1024 f]
        blk = np.asarray(results[c]["out"])
        out[b, h * SQ:(h + 1) * SQ] = (
            blk.transpose(1, 3, 0, 2).reshape(SQ, D))
    return out


def kernel(x, Wq, bq, Wk, bk, Wv, bv, Wo, bo, **kwargs):
    nc = _get_nc()
    in_maps = make_in_maps(x, Wq, bq, Wk, bk, Wv, bv, Wo, bo)
    res = bass_utils.run_bass_kernel_spmd(nc, in_maps, core_ids=list(range(N_CORES)))
    return assemble(res.results)


# revision 44
# speedup vs baseline: 1.0100x; 1.0100x over previous
"""Single-head attention (nn_MultiHeadAttention) Trainium2 Bass kernel.

Full inputs: x [4, 2048, 1024], Wq/Wk/Wv/Wo [1024, 1024], biases [1024].
reference:  q = x @ Wq.T + bq ; k,v likewise
            scores = (q @ k.T) / sqrt(1024) ; attn = softmax(scores, -1)
            out = (attn @ v) @ Wo.T + bo

Sharding: 8 cores = 4 batches x 2 query-halves; each core owns 1024
queries and all 2048 keys of its batch (global key order everywhere).

Algebraic fusions (host-side weight transforms):
  scores:  q k^T = x (Wq^T Wk) x^T + (bq Wk) x^T + per-query consts that
           cancel in softmax.  A = Wq^T Wk is precomputed on the host, so
           the K projection disappears; the per-key offset o_k = x_k.(bq Wk)
           rides in through the exp's per-partition bias.
  output:  (attn @ (x Wv^T + bv)) Wo^T + bo = attn @ (x (Wo Wv)^T + bc)
           with bc = Wo bv + bo, because the softmax rows sum to 1.  With
           Wvo = Wo Wv precomputed on the host, the ctx matmul yields the
           FINAL output directly — no separate out-projection phase.

V dedup: each core projects VO' = x (Wo Wv)^T + bc only for its OWN 1024
keys (which equal its own query rows, passed as the separate xq input so
the program stays SPMD-uniform), then the core pair exchanges halves via
a pairwise AllGather through a DRAM bounce, hidden behind QA + scores.

Per-core pipeline (all matmul operands bf16, fp32 PSUM accumulation):
  VO phase:  VO'[s,f]  = xq^T Wvo^T + bc     (own keys, d-outer, first)
             spill -> AllGather[pair] -> reload full VO'   (async)
  QA phase:  QAT[d',q] = A^T xq^T            (d-outer)
  scores:    u[k,q]    = exp((QAT^T x)^T * scale + o_k * scale)
             Z[q]      = sum_k u             (DVE accumulation + gpsimd
                                              cross-partition all-reduce)
  out:       out[f,q]  = (VO'^T u) * (1/Z)   (stored f-major, host untiles)
"""

import numpy as np
from contextlib import ExitStack

import ml_dtypes

import concourse.bass as bass
import concourse.bacc as bacc
import concourse.bass_isa as bass_isa
import concourse.mybir as mybir
import concourse.tile as tile
from concourse import bass_utils

F32 = mybir.dt.float32
F32R = mybir.dt.float32r
BF16 = mybir.dt.bfloat16
AF = mybir.ActivationFunctionType
ALU = mybir.AluOpType

B, S, D = 4, 2048, 1024
SQ = S // 2  # queries per core
N_CORES = 8
SCALE = 1.0 / float(np.sqrt(D))

# matmul operand dtypes (PSUM accumulation is always fp32)
G1DT = BF16   # x, A, qa, wvo  (QA / scores / VO matmuls)
G2DT = BF16   # vo, u          (ctx matmuls)


def build_nc():
    P = 128
    DT = D // P          # contraction tiles (8)
    ET = D // P          # output-dim tiles (8)
    SQW = 512            # query free-dim block
    SQB = SQ // SQW      # (2)
    SKT = S // P         # key tiles (16)
    SOT = SQ // P        # own-key tiles (8)
    NBW = 512            # free-dim block over D for the VO phase
    NB = D // NBW        # (2)

    nc = bacc.Bacc("TRN2", target_bir_lowering=False, debug=False,
                   num_devices=N_CORES)

    # all inputs pre-tiled on the host so every DMA chunk is one contiguous
    # DRAM run (strided row-chunks cap DMA throughput on descriptor overhead)
    xTt = nc.dram_tensor("xTt", [DT, P, S], G1DT, kind="ExternalInput")
    xqt = nc.dram_tensor("xqt", [DT, P, SQ], G1DT, kind="ExternalInput")
    aMt = nc.dram_tensor("aMt", [DT, P, D], G1DT, kind="ExternalInput")
    wvoTt = nc.dram_tensor("wvoTt", [DT, P, D], G1DT, kind="ExternalInput")
    bcd = nc.dram_tensor("bc", [D], F32, kind="ExternalInput")
    soffd = nc.dram_tensor("soff", [S], F32, kind="ExternalInput")
    outd = nc.dram_tensor("out", [ET, SQB, P, SQW], F32, kind="ExternalOutput")

    def bcast_ap(handle):
        a = handle[:]
        return bass.AP(tensor=a.tensor, offset=a.offset, ap=[[0, P]] + list(a.ap))

    with tile.TileContext(nc) as tc, ExitStack() as top:
        psum = top.enter_context(tc.tile_pool(name="psum", bufs=8, space="PSUM"))
        dram = top.enter_context(tc.tile_pool(name="dram", bufs=1, space="DRAM"))
        singles = top.enter_context(tc.tile_pool(name="singles", bufs=1))
        vb_in = dram.tile([SOT, P, D], G2DT, name="vb_in", tag="vb_in")
        vb_out = dram.tile([2, SOT, P, D], G2DT, name="vb_out", tag="vb_out")

        # ---- right-side pools, reserved in release order (LIFO top last)
        v_pool = tc.alloc_tile_pool(name="v", bufs=SKT, side="right")
        v_tiles = [v_pool.tile([P, D], G2DT, name=f"v{i}", tag="v")
                   for i in range(SKT)]
        u_pool = tc.alloc_tile_pool(name="u", bufs=SKT * SQB, side="right")
        u_tiles = [[None] * SKT for _ in range(SQB)]
        zacc_pool = tc.alloc_tile_pool(name="zacc", bufs=SQB, side="right")
        wv_pool = tc.alloc_tile_pool(name="wv", bufs=1, side="right")
        vown_pool = tc.alloc_tile_pool(name="vown", bufs=SOT, side="right")

        # ---- left-side: xt/xq under qa under a_row (released in reverse)
        xt_pool = tc.alloc_tile_pool(name="xt", bufs=DT)
        xq_pool = tc.alloc_tile_pool(name="xq", bufs=DT)
        qa_pool = tc.alloc_tile_pool(name="qa", bufs=ET)
        qa_tiles = [qa_pool.tile([P, SQ], G1DT, name=f"qa{i}", tag="qa")
                    for i in range(ET)]
        a_pool = tc.alloc_tile_pool(name="arow", bufs=DT)

        # HAM pre-warm: the PE's clock gate sits at 4/8 until ~3.4us of
        # sustained matmul activity.  Short dummy matmuls on a small zeroed
        # scratch keep the array busy through the initial DMA wait so the
        # real stream starts at full clock (its first ~13 matmuls otherwise
        # run at half rate).  The memset is gpsimd's FIRST instruction (the
        # earliest-ready engine, ~0.1us before its first DMA issue); the
        # dummy count is sized to end just as the first input tiles land.
        warm_l = singles.tile([P, P], G1DT, name="warm_l", tag="warm_l")
        nc.gpsimd.memset(warm_l, 0.0)
        wpsum = psum.tile([P, P], F32, name="mm", tag="mm")
        for _ in range(90):
            nc.tensor.matmul(wpsum, lhsT=warm_l, rhs=warm_l,
                             start=True, stop=True)

        # DMA plan: scalar stays free for ACT work (PSUM evacuation); sync
        # carries xq -> xt -> VO spills; gpsimd carries wvo -> bc -> a ->
        # the AllGather -> VO reloads -> z round-trip.  Loads are emitted in
        # consumption order: the VO d-loop of sgroup 0 needs wvo[d] +
        # xq[d][:, 0:512] per d-step.
        wv_full = wv_pool.tile([P, DT, D], G1DT, name="wv", tag="wv")
        xq_tiles = []
        for d in range(DT):
            xq_t = xq_pool.tile([P, SQ], G1DT, name=f"xq{d}", tag="xq")
            nc.sync.dma_start(out=xq_t, in_=xqt[d])
            nc.gpsimd.dma_start(out=wv_full[:, d, :], in_=wvoTt[d])
            xq_tiles.append(xq_t)
        bc_bc = singles.tile([P, D], F32, name="bc_bc", tag="bc_bc")
        nc.gpsimd.dma_start(out=bc_bc, in_=bcast_ap(bcd))
        a_rows = []
        for d in range(DT):
            ar = a_pool.tile([P, D], G1DT, name=f"ar{d}", tag="ar")
            nc.gpsimd.dma_start(out=ar, in_=aMt[d])
            a_rows.append(ar)
        xt_tiles = []
        for t in range(DT):
            xt_t = xt_pool.tile([P, S], G1DT, name=f"xt{t}", tag="xt")
            nc.sync.dma_start(out=xt_t, in_=xTt[t])
            xt_tiles.append(xt_t)

        def xt_slice(d, lo, width):
            return xt_tiles[d][:, lo:lo + width]

        # constants (emitted after the start-critical loads)
        soff_pt = singles.tile([P, SKT], F32, name="soff_pt", tag="soff_pt")
        nc.gpsimd.dma_start(out=soff_pt, in_=soffd[:].rearrange("(t p) -> p t", p=P))
        rz_bc = singles.tile([P, SQ], F32, name="rz_bc", tag="rz_bc")



        # ---------------- VO phase first (own keys only, d-outer) ----------
        # so the pair exchange starts early and hides behind QA + scores
        vown_tiles = [vown_pool.tile([P, D], G2DT, name=f"vo{i}", tag="vo")
                      for i in range(SOT)]
        for sg in range(2):
            pv = [psum.tile([P, NBW], F32, name="mm", tag="mm") for _ in range(8)]
            for d in range(DT):
                for si in range(4):
                    for eb in range(NB):
                        nc.tensor.matmul(
                            pv[si * 2 + eb],
                            lhsT=xq_tiles[d][:, (sg * 4 + si) * P:(sg * 4 + si + 1) * P],
                            rhs=wv_full[:, d, eb * NBW:(eb + 1) * NBW],
                            start=(d == 0), stop=(d == DT - 1),
                        )
            for si in range(4):
                for eb in range(NB):
                    s = sg * 4 + si
                    nc.vector.scalar_tensor_tensor(
                        out=vown_tiles[s][:, eb * NBW:(eb + 1) * NBW],
                        in0=pv[si * 2 + eb], scalar=1.0,
                        in1=bc_bc[:, eb * NBW:(eb + 1) * NBW],
                        op0=ALU.mult, op1=ALU.add,
                    )
            for si in range(4):
                s = sg * 4 + si
                nc.sync.dma_start(out=vb_in[s], in_=vown_tiles[s])
        # pairwise exchange: AllGather the spilled halves, reload both.
        nc.gpsimd.collective_compute(
            "AllGather",
            ALU.bypass,
            replica_groups=[[0, 1], [2, 3], [4, 5], [6, 7]],
            ins=[vb_in[:]],
            outs=[vb_out[:]],
        )
        for sk in range(SKT):
            nc.gpsimd.dma_start(out=v_tiles[sk], in_=vb_out[sk // SOT, sk % SOT])

        # ---------------- QA phase (d-outer) ----------------
        for sb in range(SQB):
            pq = [psum.tile([P, SQW], F32, name="mm", tag="mm") for _ in range(ET)]
            for d in range(DT):
                for et in range(ET):
                    nc.tensor.matmul(
                        pq[et],
                        lhsT=a_rows[d][:, et * P:(et + 1) * P],
                        rhs=xq_tiles[d][:, sb * SQW:(sb + 1) * SQW],
                        start=(d == 0), stop=(d == DT - 1),
                    )
            for et in range(ET):
                nc.scalar.activation(
                    out=qa_tiles[et][:, sb * SQW:(sb + 1) * SQW],
                    in_=pq[et], func=AF.Copy,
                )
        a_pool.release()

        # ---------------- scores + Z ----------------
        for sk in range(SKT):
            for q in range(SQB):
                ps = psum.tile([P, SQW], F32, name="mm", tag="mm")
                for e in range(ET):
                    nc.tensor.matmul(
                        ps,
                        lhsT=xt_slice(e, sk * P, P),
                        rhs=qa_tiles[e][:, q * SQW:(q + 1) * SQW],
                        start=(e == 0), stop=(e == ET - 1),
                    )
                ut = u_pool.tile([P, SQW], G2DT, name=f"u{q}_{sk}", tag="u")
                nc.scalar.activation(
                    out=ut, in_=ps, func=AF.Exp,
                    bias=soff_pt[:, sk:sk + 1], scale=SCALE,
                )
                u_tiles[q][sk] = ut
                if sk == 0:
                    za = zacc_pool.tile([P, SQW], F32R, name=f"za{q}", tag="za")
                    nc.vector.tensor_copy(za, ut)
                    if q == 0:
                        zacc = [za]
                    else:
                        zacc.append(za)
                else:
                    nc.vector.tensor_tensor(
                        out=zacc[q], in0=zacc[q], in1=ut, op=ALU.add)

        # Z -> 1/Z replicated across partitions, entirely off the PE queue:
        # gpsimd cross-partition all-reduce, then a DVE reciprocal
        for zq in range(SQB):
            zsum = singles.tile([P, SQW], F32, name=f"zsum{zq}", tag=f"zsum{zq}")
            nc.gpsimd.partition_all_reduce(
                zsum[:], zacc[zq][:], P, bass_isa.ReduceOp.add)
            nc.vector.reciprocal(
                out=rz_bc[:, zq * SQW:(zq + 1) * SQW], in_=zsum)

        vown_pool.release()
        wv_pool.release()
        zacc_pool.release()
        qa_pool.release()
        xq_pool.release()
        xt_pool.release()

        # ---------------- fused ctx/out phase ----------------
        with tc.tile_pool(name="ofly", bufs=4) as o_pool:
            for q in range(SQB):
                for e in range(ET):
                    pc = psum.tile([P, SQW], F32, name="mm", tag="mm")
                    for sk in range(SKT):
                        nc.tensor.matmul(
                            pc,
                            lhsT=v_tiles[sk][:, e * P:(e + 1) * P],
                            rhs=u_tiles[q][sk],
                            start=(sk == 0), stop=(sk == SKT - 1),
                        )
                    osb = o_pool.tile([P, SQW], F32, name="osb", tag="ofly")
                    last = (q == SQB - 1 and e == ET - 1)
                    # split the final block across two store queues so the
                    # kernel's last store drains sooner
                    hs = [(0, SQW // 2, nc.scalar), (SQW // 2, SQW, nc.sync)] \
                        if last else [(0, SQW, nc.scalar)]
                    for lo, hi, eng in hs:
                        nc.vector.tensor_tensor(
                            out=osb[:, lo:hi], in0=pc[:, lo:hi],
                            in1=rz_bc[:, q * SQW + lo:q * SQW + hi],
                            op=ALU.mult)
                        eng.dma_start(out=outd[e, q, :, lo:hi],
                                      in_=osb[:, lo:hi])
        u_pool.release()
        v_pool.release()

    nc.compile()
    return nc


_NC_CACHE = {}


def _get_nc():
    if "nc" not in _NC_CACHE:
        _NC_CACHE["nc"] = build_nc()
    return _NC_CACHE["nc"]


def _round_f32r(a):
    """Round-to-nearest to fp32r precision (fp22 = s1e8m13)."""
    u = np.ascontiguousarray(a, np.float32).view(np.uint32)
    u = ((u.astype(np.uint64) + 0x200) & 0xFFFFFC00).astype(np.uint32)
    return u.view(np.float32)


def _cast(a, dt):
    a = np.ascontiguousarray(np.asarray(a, np.float32))
    if dt == BF16:
        return a.astype(ml_dtypes.bfloat16)
    if dt == F32R:
        return _round_f32r(a)
    return a


def _tile_rows(m, dt):
    """[D, N] -> contiguous [D//128, 128, N] row-tiles, cast to dt."""
    m = np.asarray(m, np.float32)
    return np.ascontiguousarray(_cast(m, dt).reshape(m.shape[0] // 128, 128, -1))


def make_in_maps(x, Wq, bq, Wk, bk, Wv, bv, Wo, bo):
    x = np.asarray(x, np.float32)
    Wq = np.asarray(Wq, np.float32)
    Wk = np.asarray(Wk, np.float32)
    Wv = np.asarray(Wv, np.float32)
    Wo = np.asarray(Wo, np.float32)
    # A = Wq^T Wk so scores = x A x^T (+ per-key offset from bq, see header)
    aMt = _tile_rows(Wq.T @ Wk, G1DT)
    # Wvo = Wo Wv folds the output projection into the value path; the
    # matching bias constant is bc = Wo bv + bo (softmax rows sum to 1)
    wvoTt = _tile_rows((Wo @ Wv).T, G1DT)
    bc = np.ascontiguousarray(Wo @ np.asarray(bv, np.float32)
                              + np.asarray(bo, np.float32))
    ck = np.asarray(bq, np.float32) @ Wk  # [d]

    in_maps = []
    for c in range(N_CORES):
        b, h = c // 2, c % 2
        xb = x[b]  # [S, D], global key order
        own = xb[h * SQ:(h + 1) * SQ]
        xTt_c = _tile_rows(xb.T, G1DT)
        xqt_c = _tile_rows(own.T, G1DT)
        soff = np.ascontiguousarray((xb @ ck) * np.float32(SCALE))
        in_maps.append({
            "xTt": xTt_c, "xqt": xqt_c, "aMt": aMt, "wvoTt": wvoTt,
            "bc": bc, "soff": soff,
        })
    return in_maps


def assemble(results):
    out = np.empty((B, S, D), np.float32)
    for c in range(N_CORES):
        b, h = c // 2, c % 2
        # [8(e), 2(qb), 128(f), 512(q)] tiled, f-major -> [1024 q, 1024 f]
        blk = np.asarray(results[c]["out"])
        out[b, h * SQ:(h + 1) * SQ] = (
            blk.transpose(1, 3, 0, 2).reshape(SQ, D))
    return out


def kernel(x, Wq, bq, Wk, bk, Wv, bv, Wo, bo, **kwargs):
    nc = _get_nc()
    in_maps = make_in_maps(x, Wq, bq, Wk, bk, Wv, bv, Wo, bo)
    res = bass_utils.run_bass_kernel_spmd(nc, in_maps, core_ids=list(range(N_CORES)))
    return assemble(res.results)


# revision 46
# speedup vs baseline: 1.0172x; 1.0072x over previous
"""Single-head attention (nn_MultiHeadAttention) Trainium2 Bass kernel.

Full inputs: x [4, 2048, 1024], Wq/Wk/Wv/Wo [1024, 1024], biases [1024].
reference:  q = x @ Wq.T + bq ; k,v likewise
            scores = (q @ k.T) / sqrt(1024) ; attn = softmax(scores, -1)
            out = (attn @ v) @ Wo.T + bo

Sharding: 8 cores = 4 batches x 2 query-halves; each core owns 1024
queries and all 2048 keys of its batch (global key order everywhere).

Algebraic fusions (host-side weight transforms):
  scores:  q k^T = x (Wq^T Wk) x^T + (bq Wk) x^T + per-query consts that
           cancel in softmax.  A = Wq^T Wk is precomputed on the host, so
           the K projection disappears; the per-key offset o_k = x_k.(bq Wk)
           rides in through the exp's per-partition bias.
  output:  (attn @ (x Wv^T + bv)) Wo^T + bo = attn @ (x (Wo Wv)^T + bc)
           with bc = Wo bv + bo, because the softmax rows sum to 1.  With
           Wvo = Wo Wv precomputed on the host, the ctx matmul yields the
           FINAL output directly — no separate out-projection phase.

V dedup: each core projects VO' = x (Wo Wv)^T + bc only for its OWN 1024
keys (which equal its own query rows, passed as the separate xq input so
the program stays SPMD-uniform), then the core pair exchanges halves via
a pairwise AllGather through a DRAM bounce, hidden behind QA + scores.

Per-core pipeline (all matmul operands bf16, fp32 PSUM accumulation):
  VO phase:  VO'[s,f]  = xq^T Wvo^T + bc     (own keys, d-outer, first)
             spill -> AllGather[pair] -> reload full VO'   (async)
  QA phase:  QAT[d',q] = A^T xq^T            (d-outer)
  scores:    u[k,q]    = exp((QAT^T x)^T * scale + o_k * scale)
             Z[q]      = sum_k u             (DVE accumulation + gpsimd
                                              cross-partition all-reduce)
  out:       out[f,q]  = (VO'^T u) * (1/Z)   (stored f-major, host untiles)
"""

import numpy as np
from contextlib import ExitStack

import ml_dtypes

import concourse.bass as bass
import concourse.bacc as bacc
import concourse.bass_isa as bass_isa
import concourse.mybir as mybir
import concourse.tile as tile
from concourse import bass_utils

F32 = mybir.dt.float32
F32R = mybir.dt.float32r
BF16 = mybir.dt.bfloat16
AF = mybir.ActivationFunctionType
ALU = mybir.AluOpType

B, S, D = 4, 2048, 1024
SQ = S // 2  # queries per core
N_CORES = 8
SCALE = 1.0 / float(np.sqrt(D))

# matmul operand dtypes (PSUM accumulation is always fp32)
G1DT = BF16   # x, A, qa, wvo  (QA / scores / VO matmuls)
G2DT = BF16   # vo, u          (ctx matmuls)


def build_nc():
    P = 128
    DT = D // P          # contraction tiles (8)
    ET = D // P          # output-dim tiles (8)
    SQW = 512            # query free-dim block
    SQB = SQ // SQW      # (2)
    SKT = S // P         # key tiles (16)
    SOT = SQ // P        # own-key tiles (8)
    NBW = 512            # free-dim block over D for the VO phase
    NB = D // NBW        # (2)

    nc = bacc.Bacc("TRN2", target_bir_lowering=False, debug=False,
                   num_devices=N_CORES)

    # all inputs pre-tiled on the host so every DMA chunk is one contiguous
    # DRAM run (strided row-chunks cap DMA throughput on descriptor overhead)
    xTt = nc.dram_tensor("xTt", [DT, P, S], G1DT, kind="ExternalInput")
    xqt = nc.dram_tensor("xqt", [DT, P, SQ], G1DT, kind="ExternalInput")
    aMt = nc.dram_tensor("aMt", [DT, P, D], G1DT, kind="ExternalInput")
    wvoTt = nc.dram_tensor("wvoTt", [DT, P, D], G1DT, kind="ExternalInput")
    bcd = nc.dram_tensor("bc", [D], F32, kind="ExternalInput")
    soffd = nc.dram_tensor("soff", [S], F32, kind="ExternalInput")
    outd = nc.dram_tensor("out", [ET, SQB, P, SQW], F32, kind="ExternalOutput")

    def bcast_ap(handle):
        a = handle[:]
        return bass.AP(tensor=a.tensor, offset=a.offset, ap=[[0, P]] + list(a.ap))

    with tile.TileContext(nc) as tc, ExitStack() as top:
        psum = top.enter_context(tc.tile_pool(name="psum", bufs=8, space="PSUM"))
        dram = top.enter_context(tc.tile_pool(name="dram", bufs=1, space="DRAM"))
        singles = top.enter_context(tc.tile_pool(name="singles", bufs=1))
        vb_in = dram.tile([SOT, P, D], G2DT, name="vb_in", tag="vb_in")
        vb_out = dram.tile([2, SOT, P, D], G2DT, name="vb_out", tag="vb_out")

        # ---- right-side pools, reserved in release order (LIFO top last)
        v_pool = tc.alloc_tile_pool(name="v", bufs=SKT, side="right")
        v_tiles = [v_pool.tile([P, D], G2DT, name=f"v{i}", tag="v")
                   for i in range(SKT)]
        u_pool = tc.alloc_tile_pool(name="u", bufs=SKT * SQB, side="right")
        u_tiles = [[None] * SKT for _ in range(SQB)]
        zacc_pool = tc.alloc_tile_pool(name="zacc", bufs=SQB, side="right")
        wv_pool = tc.alloc_tile_pool(name="wv", bufs=1, side="right")
        vown_pool = tc.alloc_tile_pool(name="vown", bufs=SOT, side="right")

        # ---- left-side: xt/xq under qa under a_row (released in reverse)
        xt_pool = tc.alloc_tile_pool(name="xt", bufs=DT)
        xq_pool = tc.alloc_tile_pool(name="xq", bufs=DT)
        qa_pool = tc.alloc_tile_pool(name="qa", bufs=ET)
        qa_tiles = [qa_pool.tile([P, SQ], G1DT, name=f"qa{i}", tag="qa")
                    for i in range(ET)]
        a_pool = tc.alloc_tile_pool(name="arow", bufs=DT)

        # HAM pre-warm: the PE's clock gate sits at 4/8 until ~3.4us of
        # sustained matmul activity.  Short dummy matmuls on a small zeroed
        # scratch keep the array busy through the initial DMA wait so the
        # real stream starts at full clock (its first ~13 matmuls otherwise
        # run at half rate).  The memset is gpsimd's FIRST instruction (the
        # earliest-ready engine, ~0.1us before its first DMA issue); the
        # dummy count is sized to end just as the first input tiles land.
        warm_l = singles.tile([P, P], G1DT, name="warm_l", tag="warm_l")
        nc.gpsimd.memset(warm_l, 0.0)
        wpsum = psum.tile([P, P], F32, name="mm", tag="mm")
        for _ in range(45):
            nc.tensor.matmul(wpsum, lhsT=warm_l, rhs=warm_l,
                             start=True, stop=True)

        # DMA plan: scalar stays free for ACT work (PSUM evacuation); sync
        # carries xq -> xt -> VO spills; gpsimd carries wvo -> bc -> a ->
        # the AllGather -> VO reloads -> z round-trip.  Loads are emitted in
        # consumption order: the VO d-loop of sgroup 0 needs wvo[d] +
        # xq[d][:, 0:512] per d-step.
        wv_full = wv_pool.tile([P, DT, D], G1DT, name="wv", tag="wv")
        xq_tiles = []
        for d in range(DT):
            xq_t = xq_pool.tile([P, SQ], G1DT, name=f"xq{d}", tag="xq")
            nc.sync.dma_start(out=xq_t, in_=xqt[d])
            nc.gpsimd.dma_start(out=wv_full[:, d, :], in_=wvoTt[d])
            xq_tiles.append(xq_t)
        bc_bc = singles.tile([P, D], F32, name="bc_bc", tag="bc_bc")
        nc.gpsimd.dma_start(out=bc_bc, in_=bcast_ap(bcd))
        a_rows = []
        for d in range(DT):
            ar = a_pool.tile([P, D], G1DT, name=f"ar{d}", tag="ar")
            nc.gpsimd.dma_start(out=ar, in_=aMt[d])
            a_rows.append(ar)
        xt_tiles = []
        for t in range(DT):
            xt_t = xt_pool.tile([P, S], G1DT, name=f"xt{t}", tag="xt")
            nc.sync.dma_start(out=xt_t, in_=xTt[t])
            xt_tiles.append(xt_t)

        def xt_slice(d, lo, width):
            return xt_tiles[d][:, lo:lo + width]

        # constants (emitted after the start-critical loads)
        soff_pt = singles.tile([P, SKT], F32, name="soff_pt", tag="soff_pt")
        nc.gpsimd.dma_start(out=soff_pt, in_=soffd[:].rearrange("(t p) -> p t", p=P))
        rz_bc = singles.tile([P, SQ], F32, name="rz_bc", tag="rz_bc")



        # ---------------- VO phase first (own keys only, d-outer) ----------
        # so the pair exchange starts early and hides behind QA + scores
        vown_tiles = [vown_pool.tile([P, D], G2DT, name=f"vo{i}", tag="vo")
                      for i in range(SOT)]
        for sg in range(2):
            pv = [psum.tile([P, NBW], F32, name="mm", tag="mm") for _ in range(8)]
            for d in range(DT):
                for si in range(4):
                    for eb in range(NB):
                        nc.tensor.matmul(
                            pv[si * 2 + eb],
                            lhsT=xq_tiles[d][:, (sg * 4 + si) * P:(sg * 4 + si + 1) * P],
                            rhs=wv_full[:, d, eb * NBW:(eb + 1) * NBW],
                            start=(d == 0), stop=(d == DT - 1),
                        )
            for si in range(4):
                for eb in range(NB):
                    s = sg * 4 + si
                    nc.vector.scalar_tensor_tensor(
                        out=vown_tiles[s][:, eb * NBW:(eb + 1) * NBW],
                        in0=pv[si * 2 + eb], scalar=1.0,
                        in1=bc_bc[:, eb * NBW:(eb + 1) * NBW],
                        op0=ALU.mult, op1=ALU.add,
                    )
            for si in range(4):
                s = sg * 4 + si
                nc.sync.dma_start(out=vb_in[s], in_=vown_tiles[s])
        # pairwise exchange: AllGather the spilled halves, reload both.
        nc.gpsimd.collective_compute(
            "AllGather",
            ALU.bypass,
            replica_groups=[[0, 1], [2, 3], [4, 5], [6, 7]],
            ins=[vb_in[:]],
            outs=[vb_out[:]],
        )
        for sk in range(SKT):
            nc.gpsimd.dma_start(out=v_tiles[sk], in_=vb_out[sk // SOT, sk % SOT])

        # ---------------- QA phase (d-outer) ----------------
        for sb in range(SQB):
            pq = [psum.tile([P, SQW], F32, name="mm", tag="mm") for _ in range(ET)]
            for d in range(DT):
                for et in range(ET):
                    nc.tensor.matmul(
                        pq[et],
                        lhsT=a_rows[d][:, et * P:(et + 1) * P],
                        rhs=xq_tiles[d][:, sb * SQW:(sb + 1) * SQW],
                        start=(d == 0), stop=(d == DT - 1),
                    )
            for et in range(ET):
                nc.scalar.activation(
                    out=qa_tiles[et][:, sb * SQW:(sb + 1) * SQW],
                    in_=pq[et], func=AF.Copy,
                )
        a_pool.release()

        # ---------------- scores + Z ----------------
        for sk in range(SKT):
            for q in range(SQB):
                ps = psum.tile([P, SQW], F32, name="mm", tag="mm")
                for e in range(ET):
                    nc.tensor.matmul(
                        ps,
                        lhsT=xt_slice(e, sk * P, P),
                        rhs=qa_tiles[e][:, q * SQW:(q + 1) * SQW],
                        start=(e == 0), stop=(e == ET - 1),
                    )
                ut = u_pool.tile([P, SQW], G2DT, name=f"u{q}_{sk}", tag="u")
                nc.scalar.activation(
                    out=ut, in_=ps, func=AF.Exp,
                    bias=soff_pt[:, sk:sk + 1], scale=SCALE,
                )
                u_tiles[q][sk] = ut
                if sk == 0:
                    za = zacc_pool.tile([P, SQW], F32R, name=f"za{q}", tag="za")
                    nc.vector.tensor_copy(za, ut)
                    if q == 0:
                        zacc = [za]
                    else:
                        zacc.append(za)
                else:
                    nc.vector.tensor_tensor(
                        out=zacc[q], in0=zacc[q], in1=ut, op=ALU.add)

        # Z -> 1/Z replicated across partitions, entirely off the PE queue:
        # gpsimd cross-partition all-reduce, then a DVE reciprocal
        for zq in range(SQB):
            zsum = singles.tile([P, SQW], F32, name=f"zsum{zq}", tag=f"zsum{zq}")
            nc.gpsimd.partition_all_reduce(
                zsum[:], zacc[zq][:], P, bass_isa.ReduceOp.add)
            nc.vector.reciprocal(
                out=rz_bc[:, zq * SQW:(zq + 1) * SQW], in_=zsum)

        vown_pool.release()
        wv_pool.release()
        zacc_pool.release()
        qa_pool.release()
        xq_pool.release()
        xt_pool.release()

        # ---------------- fused ctx/out phase ----------------
        with tc.tile_pool(name="ofly", bufs=4) as o_pool:
            for q in range(SQB):
                for e in range(ET):
                    pc = psum.tile([P, SQW], F32, name="mm", tag="mm")
                    for sk in range(SKT):
                        nc.tensor.matmul(
                            pc,
                            lhsT=v_tiles[sk][:, e * P:(e + 1) * P],
                            rhs=u_tiles[q][sk],
                            start=(sk == 0), stop=(sk == SKT - 1),
                        )
                    osb = o_pool.tile([P, SQW], F32, name="osb", tag="ofly")
                    nc.vector.tensor_tensor(
                        out=osb, in0=pc,
                        in1=rz_bc[:, q * SQW:(q + 1) * SQW], op=ALU.mult)
                    nc.scalar.dma_start(out=outd[e, q], in_=osb)
        u_pool.release()
        v_pool.release()

    nc.compile()
    return nc


_NC_CACHE = {}


def _get_nc():
    if "nc" not in _NC_CACHE:
        _NC_CACHE["nc"] = build_nc()
    return _NC_CACHE["nc"]


def _round_f32r(a):
    """Round-to-nearest to fp32r precision (fp22 = s1e8m13)."""
    u = np.ascontiguousarray(a, np.float32).view(np.uint32)
    u = ((u.astype(np.uint64) + 0x200) & 0xFFFFFC00).astype(np.uint32)
    return u.view(np.float32)


def _cast(a, dt):
    a = np.ascontiguousarray(np.asarray(a, np.float32))
    if dt == BF16:
        return a.astype(ml_dtypes.bfloat16)
    if dt == F32R:
        return _round_f32r(a)
    return a


def _tile_rows(m, dt):
    """[D, N] -> contiguous [D//128, 128, N] row-tiles, cast to dt."""
    m = np.asarray(m, np.float32)
    return np.ascontiguousarray(_cast(m, dt).reshape(m.shape[0] // 128, 128, -1))


def make_in_maps(x, Wq, bq, Wk, bk, Wv, bv, Wo, bo):
    x = np.asarray(x, np.float32)
    Wq = np.asarray(Wq, np.float32)
    Wk = np.asarray(Wk, np.float32)
    Wv = np.asarray(Wv, np.float32)
    Wo = np.asarray(Wo, np.float32)
    # A = Wq^T Wk so scores = x A x^T (+ per-key offset from bq, see header)
    aMt = _tile_rows(Wq.T @ Wk, G1DT)
    # Wvo = Wo Wv folds the output projection into the value path; the
    # matching bias constant is bc = Wo bv + bo (softmax rows sum to 1)
    wvoTt = _tile_rows((Wo @ Wv).T, G1DT)
    bc = np.ascontiguousarray(Wo @ np.asarray(bv, np.float32)
                              + np.asarray(bo, np.float32))
    ck = np.asarray(bq, np.float32) @ Wk  # [d]

    in_maps = []
    for c in range(N_CORES):
        b, h = c // 2, c % 2
        xb = x[b]  # [S, D], global key order
        own = xb[h * SQ:(h + 1) * SQ]
        xTt_c = _tile_rows(xb.T, G1DT)
        xqt_c = _tile_rows(own.T, G1DT)
        soff = np.ascontiguousarray((xb @ ck) * np.float32(SCALE))
        in_maps.append({
            "xTt": xTt_c, "xqt": xqt_c, "aMt": aMt, "wvoTt": wvoTt,
            "bc": bc, "soff": soff,
        })
    return in_maps


def assemble(results):
    out = np.empty((B, S, D), np.float32)
    for c in range(N_CORES):
        b, h = c // 2, c % 2
        # [8(e), 2(qb), 128(f), 512(q)] tiled, f-major -> [1024 q, 1024 f]
        blk = np.asarray(results[c]["out"])
        out[b, h * SQ:(h + 1) * SQ] = (
            blk.transpose(1, 3, 0, 2).reshape(SQ, D))
    return out


def kernel(x, Wq, bq, Wk, bk, Wv, bv, Wo, bo, **kwargs):
    nc = _get_nc()
    in_maps = make_in_maps(x, Wq, bq, Wk, bk, Wv, bv, Wo, bo)
    res = bass_utils.run_bass_kernel_spmd(nc, in_maps, core_ids=list(range(N_CORES)))
    return assemble(res.results)
